# revision 1
# baseline (speedup 1.0000x reference)
"""GQA forward (B=2,T=2048,E=2048,H=32,HKV=8,D=64, RoPE, causal) on 8 trn2 cores.

Sharding: tensor-parallel over kv-heads. Core c owns kv-head c and q-heads
4c..4c+3 (columns 256c:256c+256 of Wq, 64c:64c+64 of Wk/Wv, rows
256c:256c+256 of Wo). Each core computes its heads' attention for both
batches plus the partial o-projection y_c @ Wo_c; the host sums the 8
partials.

On-core layout: everything transposed (feature dim on partitions).
  x^T tiles  [e=128, t=512]   via DMA transpose
  Q^T        [256, 2048]/batch (2 tiles of 128 = 2 head pairs), RoPE applied
  K^T        [64, 2048]/batch, RoPE applied
  V          [t-blocks of 128, 65]  (col 64 = ones -> row 64 of y psum = softmax denom)
  scores^T   [s=128, tq<=512] = K^T_blk.T @ Q^T  (PE, contraction d=64)
  P^T        = exp(0.125*S^T) bf16, causal corner mask multiplied in
  y^T        [65, tq] += V_aug.T @ P^T  (PE, contraction s=128)
  normalize  y^T[0:64] * broadcast(1/y^T[64])  (recip DVE + ones-matmul bcast)
  out_part   [t=128, e=512] = Y^T_chunk.T @ Wo_chunk (accum over 2 chunks)
No softmax max-subtraction: scores are O(5), exp stays in fp32 range.
"""
import os

import numpy as np
import ml_dtypes

import concourse.mybir as mybir
import concourse.tile as tile
from concourse import bacc
from concourse.bass_utils import run_bass_kernel_spmd

F32 = mybir.dt.float32
BF16 = mybir.dt.bfloat16
AF = mybir.ActivationFunctionType
BF16NP = ml_dtypes.bfloat16

B, T, E = 2, 2048, 2048
H, HKV, D = 32, 8, 64
G = H // HKV          # q heads per kv head (= per core)
NCORES = 8
QH = G * D            # 256 q cols per core
ECH = E // 128        # 16 contraction chunks
TQC = 512             # tq chunk width
NTQ = T // TQC        # 4
NSB = T // 128        # 16 key blocks per batch
ROPE_BASE = 10000.0

_compiled = None
LAST_RESULT = None
DEBUG_DUMPS = False


def _build():
    nc = bacc.Bacc(None, target_bir_lowering=False, debug=False)

    x_d = nc.declare_dram_parameter("x", [B * T, E], BF16, isOutput=False)
    wq_d = nc.declare_dram_parameter("wq", [E, QH], BF16, isOutput=False)
    wk_d = nc.declare_dram_parameter("wk", [E, D], BF16, isOutput=False)
    wv_d = nc.declare_dram_parameter("wv", [E, D], BF16, isOutput=False)
    wo_d = nc.declare_dram_parameter("wo", [QH, E], BF16, isOutput=False)
    cos_d = nc.declare_dram_parameter("cos", [128, T], F32, isOutput=False)
    sin_d = nc.declare_dram_parameter("sin", [128, T], F32, isOutput=False)
    cm_d = nc.declare_dram_parameter("cmask", [128, 128], BF16, isOutput=False)
    out_d = nc.declare_dram_parameter("out", [B * T, E], BF16, isOutput=True)
    dbg = {}
    if DEBUG_DUMPS:
        dbg["qT00"] = nc.declare_dram_parameter("d_qT00", [64, T], BF16, isOutput=True)
        dbg["kT0"] = nc.declare_dram_parameter("d_kT0", [64, T], BF16, isOutput=True)
        dbg["vS0"] = nc.declare_dram_parameter("d_vS0", [128, NSB * (D + 1)], BF16, isOutput=True)
        dbg["yT00"] = nc.declare_dram_parameter("d_yT00", [128, T], BF16, isOutput=True)

    with tile.TileContext(nc) as tc:
        with (
            tc.tile_pool(name="const", bufs=1) as cp,
            tc.tile_pool(name="acts", bufs=1) as ac,
            tc.tile_pool(name="xT", bufs=2) as xp,
            tc.tile_pool(name="work", bufs=3) as wp,
            tc.tile_pool(name="psA", bufs=3, space="PSUM") as psA,
            tc.tile_pool(name="psS", bufs=2, space="PSUM") as psS,
            tc.tile_pool(name="psY", bufs=2, space="PSUM") as psY,
            tc.tile_pool(name="psB", bufs=1, space="PSUM") as psB,
        ):
            wq_sb = cp.tile([128, ECH, QH], BF16)
            nc.sync.dma_start(wq_sb[:], wq_d[:, :].rearrange("(c p) n -> p c n", p=128))
            wk_sb = cp.tile([128, ECH, D], BF16)
            nc.sync.dma_start(wk_sb[:], wk_d[:, :].rearrange("(c p) n -> p c n", p=128))
            wv_sb = cp.tile([128, ECH, D], BF16)
            nc.sync.dma_start(wv_sb[:], wv_d[:, :].rearrange("(c p) n -> p c n", p=128))
            wo_sb = cp.tile([128, 2, E], BF16)
            nc.sync.dma_start(wo_sb[:], wo_d[:, :].rearrange("(c p) n -> p c n", p=128))
            cos_sb = cp.tile([128, T], F32)
            nc.sync.dma_start(cos_sb[:], cos_d[:, :])
            sin_sb = cp.tile([128, T], F32)
            nc.sync.dma_start(sin_sb[:], sin_d[:, :])
            cm_sb = cp.tile([128, 128], BF16)
            nc.sync.dma_start(cm_sb[:], cm_d[:, :])
            ones_sb = cp.tile([1, D], BF16)
            nc.vector.memset(ones_sb[:], 1.0)

            qT, kT, vT, vS, yT = {}, {}, {}, {}, {}
            for b in range(B):
                kT[b] = ac.tile([64, T], BF16, name=f"kT{b}", tag=f"kT{b}")
                vT[b] = ac.tile([64, T], BF16, name=f"vT{b}", tag=f"vT{b}")
                for i in range(NSB):
                    vS[b, i] = ac.tile([128, D + 1], BF16, name=f"vS{b}_{i}",
                                       tag=f"vS{b}_{i}")
                    nc.vector.memset(vS[b, i][:, D:D + 1], 1.0)
                for g in range(G):
                    qT[b, g] = ac.tile([64, T], BF16, name=f"qT{b}{g}", tag=f"qT{b}{g}")
                for hp in range(2):
                    yT[b, hp] = ac.tile([128, T], BF16, name=f"yT{b}{hp}", tag=f"yT{b}{hp}")

            def rope_store(dst, psrc, p0, tc0):
                # dst[bf16][0:64, TQC] <- rope(psum[p0:p0+64, TQC]); period-64 tables
                t1 = wp.tile([64, TQC], F32, tag="t1")
                t2 = wp.tile([64, TQC], F32, tag="t2")
                nc.vector.tensor_mul(t1[:, :], psrc[p0:p0 + 64, :],
                                     cos_sb[0:64, tc0:tc0 + TQC])
                nc.vector.tensor_mul(t2[0:32, :], psrc[p0 + 32:p0 + 64, :],
                                     sin_sb[0:32, tc0:tc0 + TQC])
                nc.vector.tensor_mul(t2[32:64, :], psrc[p0:p0 + 32, :],
                                     sin_sb[32:64, tc0:tc0 + TQC])
                nc.vector.tensor_add(dst, t1[:, :], t2[:, :])

            for b in range(B):
                for j in range(NTQ):
                    rows0 = b * T + j * TQC
                    tc0 = j * TQC
                    xT_t = xp.tile([128, ECH, TQC], BF16, tag="xT")
                    for ec in range(ECH):
                        nc.sync.dma_start_transpose(
                            xT_t[:, ec, :],
                            x_d[rows0:rows0 + TQC, ec * 128:(ec + 1) * 128])
                    for hp in range(2):
                        qp = psA.tile([128, TQC], F32, tag="proj")
                        for ec in range(ECH):
                            nc.tensor.matmul(qp[:],
                                             wq_sb[:, ec, 128 * hp:128 * hp + 128],
                                             xT_t[:, ec, :],
                                             start=(ec == 0), stop=(ec == ECH - 1))
                        rope_store(qT[b, 2 * hp][:, tc0:tc0 + TQC], qp, 0, tc0)
                        rope_store(qT[b, 2 * hp + 1][:, tc0:tc0 + TQC], qp, 64, tc0)
                    kp = psA.tile([128, TQC], F32, tag="proj")
                    for ec in range(ECH):
                        nc.tensor.matmul(kp[0:64, :], wk_sb[:, ec, :], xT_t[:, ec, :],
                                         start=(ec == 0), stop=(ec == ECH - 1))
                    rope_store(kT[b][:, tc0:tc0 + TQC], kp, 0, tc0)
                    vp = psA.tile([128, TQC], F32, tag="proj")
                    for ec in range(ECH):
                        nc.tensor.matmul(vp[0:64, :], wv_sb[:, ec, :], xT_t[:, ec, :],
                                         start=(ec == 0), stop=(ec == ECH - 1))
                    nc.scalar.activation(vT[b][:, tc0:tc0 + TQC], vp[0:64, :], AF.Copy)
                    for tb in range(4):
                        sb_i = j * 4 + tb
                        nc.sync.dma_start_transpose(
                            vS[b, sb_i][:, 0:D],
                            vT[b][:, tc0 + tb * 128:tc0 + (tb + 1) * 128])

                for j in range(NTQ):
                    tc0 = j * TQC
                    for g in range(G):
                        hp, h1 = g // 2, (g % 2) * 64
                        yp = psY.tile([65, TQC], F32, tag="y")
                        nsb = 4 * j + 4
                        for sb_i in range(nsb):
                            k = sb_i - 4 * j
                            j0 = 128 * k if k >= 0 else 0
                            sp = psS.tile([128, TQC], F32, tag="s")
                            nc.tensor.matmul(sp[:, j0:TQC],
                                             kT[b][:, 128 * sb_i:128 * sb_i + 128],
                                             qT[b, g][:, tc0 + j0:tc0 + TQC],
                                             start=True, stop=True)
                            pt = wp.tile([128, TQC], BF16, tag="p")
                            nc.scalar.activation(pt[:, j0:TQC], sp[:, j0:TQC],
                                                 AF.Exp, scale=0.125)
                            if k >= 0:
                                nc.vector.tensor_mul(pt[:, j0:j0 + 128],
                                                     pt[:, j0:j0 + 128], cm_sb[:])
                            nc.tensor.matmul(yp[:, j0:TQC], vS[b, sb_i][:, :],
                                             pt[:, j0:TQC],
                                             start=(sb_i == 0), stop=(sb_i == nsb - 1),
                                             skip_group_check=True)
                        rc = wp.tile([1, TQC], BF16, tag="rc")
                        with nc.allow_low_precision(reason="softmax denom bcast via bf16 matmul"):
                            nc.vector.reciprocal(rc[:], yp[64:65, :])
                        bc = psB.tile([64, TQC], F32, tag="bc")
                        nc.tensor.matmul(bc[:], ones_sb[:], rc[:], start=True, stop=True)
                        yf = wp.tile([64, TQC], F32, tag="yf")
                        nc.any.tensor_copy(yf[:], yp[0:64, :])
                        nc.vector.tensor_mul(yT[b, hp][h1:h1 + 64, tc0:tc0 + TQC],
                                             yf[:], bc[:])
                    for tb in range(4):
                        r0 = tc0 + tb * 128
                        for ecol in range(4):
                            op = psA.tile([128, TQC], F32, tag="proj")
                            for hc in range(2):
                                nc.tensor.matmul(op[:], yT[b, hc][:, r0:r0 + 128],
                                                 wo_sb[:, hc, 512 * ecol:512 * ecol + 512],
                                                 start=(hc == 0), stop=(hc == 1))
                            ot = wp.tile([128, TQC], BF16, tag="o")
                            nc.any.tensor_copy(ot[:], op[:])
                            nc.sync.dma_start(
                                out_d[b * T + r0:b * T + r0 + 128,
                                      512 * ecol:512 * ecol + 512],
                                ot[:])

            if DEBUG_DUMPS:
                nc.sync.dma_start(dbg["qT00"][:, :], qT[0, 0][:])
                nc.sync.dma_start(dbg["kT0"][:, :], kT[0][:])
                for i in range(NSB):
                    nc.sync.dma_start(dbg["vS0"][:, i * (D + 1):(i + 1) * (D + 1)],
                                      vS[0, i][:, :])
                nc.sync.dma_start(dbg["yT00"][:, :], yT[0, 0][:])

    nc.compile()
    return nc


def _host_consts():
    inv = ROPE_BASE ** (-np.arange(32, dtype=np.float64) / 32.0)
    ang = np.outer(inv, np.arange(T, dtype=np.float64))          # [32, T]
    cos128 = np.tile(np.cos(ang), (4, 1)).astype(np.float32)
    sin32 = np.sin(ang)
    sinS = np.tile(np.concatenate([-sin32, sin32], axis=0), (2, 1)).astype(np.float32)
    cmask = np.triu(np.ones((128, 128))).astype(BF16NP)          # valid iff p <= j
    return cos128, sinS, cmask


def kernel(x, Wq, Wk, Wv, Wo):
    global _compiled, LAST_RESULT
    if _compiled is None:
        _compiled = _build()
    nc = _compiled

    xb = np.ascontiguousarray(x.reshape(B * T, E)).astype(BF16NP)
    cos128, sinS, cmask = _host_consts()
    in_maps = []
    for c in range(NCORES):
        in_maps.append({
            "x": xb,
            "wq": np.ascontiguousarray(Wq[:, QH * c:QH * (c + 1)]).astype(BF16NP),
            "wk": np.ascontiguousarray(Wk[:, D * c:D * (c + 1)]).astype(BF16NP),
            "wv": np.ascontiguousarray(Wv[:, D * c:D * (c + 1)]).astype(BF16NP),
            "wo": np.ascontiguousarray(Wo[QH * c:QH * (c + 1), :]).astype(BF16NP),
            "cos": cos128,
            "sin": sinS,
            "cmask": cmask,
        })
    trace = os.environ.get("GQA_TRACE", "0") == "1"
    res = run_bass_kernel_spmd(nc, in_maps, core_ids=list(range(NCORES)), trace=trace)
    LAST_RESULT = res
    acc = np.zeros((B * T, E), np.float32)
    for r in res.results:
        acc += np.asarray(r["out"]).astype(np.float32)
    return acc.reshape(B, T, E)



# revision 10
# speedup vs baseline: 1.4380x; 1.4380x over previous
"""GQA forward (B=2,T=2048,E=2048,H=32,HKV=8,D=64, RoPE, causal) on 8 trn2 cores.

Sharding: tensor-parallel over kv-heads. Core c owns kv-head c and q-heads
4c..4c+3 (columns 256c:256c+256 of Wq, 64c:64c+64 of Wk/Wv, rows
256c:256c+256 of Wo). Each core computes its heads' attention for both
batches plus the partial o-projection y_c @ Wo_c; the host sums the 8
partials.

v2 layout/schedule (vs v1 baseline):
  - x^T is pre-transposed on the host ([E, B*T] bf16) -> plain wide DMAs,
    no on-chip DMA transposes for x.
  - K and V projections packed into one [E,128] stationary (K^T rows 0:64,
    V^T rows 64:128 of one psum).
  - exp() batched over pairs of key blocks ([128,1024] psum reads).
  - softmax denominators for the 4 groups of a tq-chunk collected into one
    [4,512] tile via tiny DMAs; ONE reciprocal per (b,chunk) instead of 32
    pathological [1,512] reciprocals.
  - y kept unnormalized in SBUF; bcast+normalize+o-projection emitted one
    chunk late so the PE never waits on the reciprocal chain.
  - causal corner masks on gpsimd; copies split DVE/Act to balance engines.
"""
import os

import numpy as np
import ml_dtypes

import concourse.mybir as mybir
import concourse.tile as tile
from concourse import bacc
from concourse.bass_utils import run_bass_kernel_spmd

F32 = mybir.dt.float32
BF16 = mybir.dt.bfloat16
AF = mybir.ActivationFunctionType
BF16NP = ml_dtypes.bfloat16

B, T, E = 2, 2048, 2048
H, HKV, D = 32, 8, 64
G = H // HKV          # q heads per kv head (= per core)
NCORES = 8
QH = G * D            # 256 q cols per core
ECH = E // 128        # 16 contraction chunks
TQC = 512             # tq chunk width
NTQ = T // TQC        # 4
NSB = T // 128        # 16 key blocks per batch
ROPE_BASE = 10000.0

_compiled = None
LAST_RESULT = None


def _build():
    nc = bacc.Bacc(None, target_bir_lowering=False, debug=False)

    xt_d = nc.declare_dram_parameter("xT", [E, B * T], BF16, isOutput=False)
    wq_d = nc.declare_dram_parameter("wq", [E, QH], BF16, isOutput=False)
    wkv_d = nc.declare_dram_parameter("wkv", [E, 2 * D], BF16, isOutput=False)
    wo_d = nc.declare_dram_parameter("wo", [QH, E], BF16, isOutput=False)
    cos_d = nc.declare_dram_parameter("cos", [64, T], F32, isOutput=False)
    sin_d = nc.declare_dram_parameter("sin", [64, T], F32, isOutput=False)
    cm_d = nc.declare_dram_parameter("cmask", [128, 128], BF16, isOutput=False)
    out_d = nc.declare_dram_parameter("out", [B * T, E], BF16, isOutput=True)

    with tile.TileContext(nc) as tc:
        with (
            tc.tile_pool(name="const", bufs=1) as cp,
            tc.tile_pool(name="acts", bufs=1) as ac,
            tc.tile_pool(name="xT", bufs=2) as xp,
            tc.tile_pool(name="work", bufs=2) as wp,
            tc.tile_pool(name="ps1", bufs=2, space="PSUM") as ps1,   # 2 banks
            tc.tile_pool(name="psS", bufs=2, space="PSUM") as psS,   # 4 banks
            tc.tile_pool(name="psY", bufs=2, space="PSUM") as psY,   # 2 banks
        ):
            wq_sb = cp.tile([128, ECH, QH], BF16)
            nc.sync.dma_start(wq_sb[:], wq_d[:, :].rearrange("(c p) n -> p c n", p=128))
            wkv_sb = cp.tile([128, ECH, 2 * D], BF16)
            nc.sync.dma_start(wkv_sb[:], wkv_d[:, :].rearrange("(c p) n -> p c n", p=128))
            wo_sb = cp.tile([128, 2, E], BF16)
            nc.sync.dma_start(wo_sb[:], wo_d[:, :].rearrange("(c p) n -> p c n", p=128))
            cos_sb = cp.tile([64, T], F32)
            nc.sync.dma_start(cos_sb[:], cos_d[:, :])
            sin_sb = cp.tile([64, T], F32)
            nc.sync.dma_start(sin_sb[:], sin_d[:, :])
            cm_sb = cp.tile([128, 128], BF16)
            nc.sync.dma_start(cm_sb[:], cm_d[:, :])
            ones_sb = cp.tile([1, D], BF16)
            nc.vector.memset(ones_sb[:], 1.0)

            kT, vS = {}, {}
            for b in range(B):
                kT[b] = ac.tile([64, T], BF16, name=f"kT{b}", tag=f"kT{b}")
                for i in range(NSB):
                    vS[b, i] = ac.tile([128, D + 1], BF16, name=f"vS{b}_{i}",
                                       tag=f"vS{b}_{i}")
                    nc.vector.memset(vS[b, i][:, D:D + 1], 1.0)
            # unnormalized y (head-pair-major) and normalized y; column ranges
            # are disjoint across (b, j) so two shared tiles each suffice.
            yU, yN = {}, {}
            for hp in range(2):
                yU[hp] = ac.tile([128, B * T], BF16, name=f"yU{hp}", tag=f"yU{hp}")
                yN[hp] = ac.tile([128, B * T], BF16, name=f"yN{hp}", tag=f"yN{hp}")

            def rope_store(dst, psrc, p0, tc0):
                # dst[bf16][0:64, TQC] <- rope(psum[p0:p0+64, TQC]); period-64 tables
                t1 = wp.tile([64, TQC], F32, tag="t1")
                t2 = wp.tile([64, TQC], F32, tag="t2")
                nc.vector.tensor_mul(t1[:, :], psrc[p0:p0 + 64, :],
                                     cos_sb[0:64, tc0:tc0 + TQC])
                nc.vector.tensor_mul(t2[0:32, :], psrc[p0 + 32:p0 + 64, :],
                                     sin_sb[0:32, tc0:tc0 + TQC])
                nc.vector.tensor_mul(t2[32:64, :], psrc[p0:p0 + 32, :],
                                     sin_sb[32:64, tc0:tc0 + TQC])
                nc.vector.tensor_add(dst, t1[:, :], t2[:, :])

            def emit_proj(b, j):
                rows0 = b * T + j * TQC
                tc0 = j * TQC
                xT_t = xp.tile([128, ECH, TQC], BF16, tag="xT")
                nc.sync.dma_start(
                    xT_t[:],
                    xt_d[:, rows0:rows0 + TQC].rearrange("(c p) n -> p c n", p=128))
                qTc = [wp.tile([64, TQC], BF16, tag=f"qT{g}", name=f"qT{g}")
                       for g in range(G)]
                for hp in range(2):
                    qp = ps1.tile([128, TQC], F32, tag="p1")
                    for ec in range(ECH):
                        nc.tensor.matmul(qp[:],
                                         wq_sb[:, ec, 128 * hp:128 * hp + 128],
                                         xT_t[:, ec, :],
                                         start=(ec == 0), stop=(ec == ECH - 1))
                    rope_store(qTc[2 * hp][:, :], qp, 0, tc0)
                    rope_store(qTc[2 * hp + 1][:, :], qp, 64, tc0)
                kvp = psY.tile([128, TQC], F32, tag="y")
                for ec in range(ECH):
                    nc.tensor.matmul(kvp[:], wkv_sb[:, ec, :], xT_t[:, ec, :],
                                     start=(ec == 0), stop=(ec == ECH - 1))
                rope_store(kT[b][:, tc0:tc0 + TQC], kvp, 0, tc0)
                vT = wp.tile([64, TQC], BF16, tag="vT")
                nc.scalar.copy(vT[:, :], kvp[64:128, :])
                for tb in range(4):
                    nc.sync.dma_start_transpose(
                        vS[b, j * 4 + tb][:, 0:D],
                        vT[:, tb * 128:(tb + 1) * 128])
                return qTc

            def emit_attn_core(b, j, qTc):
                """scores+exp+PV for all 4 groups; collect denoms; recip."""
                tc0 = j * TQC
                # denom rows parked at quadrant-aligned partitions 0/32/64/96
                # (engine APs cannot start at arbitrary partitions)
                den = wp.tile([128, TQC], F32, tag="den")
                rc = wp.tile([128, TQC], BF16, tag="rc")
                for g in range(G):
                    hp, h1 = g // 2, (g % 2) * 64
                    yp = psY.tile([65, TQC], F32, tag="y")
                    first = True
                    # off-diagonal full key blocks, in pairs
                    for p in range(2 * j):
                        sp2 = psS.tile([128, 2, TQC], F32, tag="s2")
                        pt2 = wp.tile([128, 2, TQC], BF16, tag="p2", bufs=3)
                        for u in range(2):
                            sb = 2 * p + u
                            nc.tensor.matmul(sp2[:, u, :],
                                             kT[b][:, 128 * sb:128 * sb + 128],
                                             qTc[g][:, :],
                                             start=True, stop=True)
                        nc.scalar.activation(pt2[:, :, :], sp2[:, :, :],
                                             AF.Exp, scale=0.125)
                        for u in range(2):
                            nc.tensor.matmul(yp[:, :], vS[b, 2 * p + u][:, :],
                                             pt2[:, u, :],
                                             start=first, stop=False,
                                             skip_group_check=True)
                            first = False
                    # diagonal 4 blocks (partial widths), 2 per psum pair-tile
                    for kk in range(0, 4, 2):
                        sp2 = psS.tile([128, 2, TQC], F32, tag="s2")
                        pt2 = wp.tile([128, 2, TQC], BF16, tag="p2", bufs=3)
                        for u in range(2):
                            k = kk + u
                            j0 = 128 * k
                            nc.tensor.matmul(sp2[:, u, j0:TQC],
                                             kT[b][:, 128 * (4 * j + k):128 * (4 * j + k) + 128],
                                             qTc[g][:, j0:TQC],
                                             start=True, stop=True)
                        for u in range(2):
                            k = kk + u
                            j0 = 128 * k
                            nc.scalar.activation(pt2[:, u, j0:TQC], sp2[:, u, j0:TQC],
                                                 AF.Exp, scale=0.125)
                            nc.gpsimd.tensor_mul(pt2[:, u, j0:j0 + 128],
                                                 pt2[:, u, j0:j0 + 128], cm_sb[:])
                            nc.tensor.matmul(yp[:, j0:TQC], vS[b, 4 * j + k][:, :],
                                             pt2[:, u, j0:TQC],
                                             start=(first and k == 0), stop=(k == 3),
                                             skip_group_check=True)
                    # free psum fast: unnormalized y -> SBUF, denom row -> collector
                    nc.scalar.copy(yU[hp][h1:h1 + 64, b * T + tc0:b * T + tc0 + TQC],
                                   yp[0:64, :])
                    nc.vector.tensor_copy(den[32 * g:32 * g + 1, :], yp[64:65, :])
                with nc.allow_low_precision(reason="softmax denom bcast via bf16 matmul"):
                    nc.vector.reciprocal(rc[:, :], den[:, :])
                return rc

            def emit_norm_oproj(b, j, rc):
                tc0 = j * TQC
                c0 = b * T + tc0
                rcr = wp.tile([1, 4, TQC], BF16, tag="rcr")
                for g in range(G):
                    nc.sync.dma_start(rcr[0:1, g, :], rc[32 * g:32 * g + 1, :])
                for g in range(G):
                    hp, h1 = g // 2, (g % 2) * 64
                    bc = ps1.tile([64, TQC], F32, tag="p1")
                    nc.tensor.matmul(bc[:], ones_sb[:], rcr[0:1, g, :],
                                     start=True, stop=True)
                    nc.vector.tensor_mul(yN[hp][h1:h1 + 64, c0:c0 + TQC],
                                         yU[hp][h1:h1 + 64, c0:c0 + TQC], bc[:])
                for tb in range(4):
                    r0 = tc0 + tb * 128
                    ot = wp.tile([128, E], BF16, tag="ot")
                    for ecol in range(4):
                        op = ps1.tile([128, TQC], F32, tag="p1")
                        for hc in range(2):
                            nc.tensor.matmul(op[:], yN[hc][:, b * T + r0:b * T + r0 + 128],
                                             wo_sb[:, hc, 512 * ecol:512 * ecol + 512],
                                             start=(hc == 0), stop=(hc == 1))
                        if ecol % 2 == 0:
                            nc.vector.tensor_copy(ot[:, 512 * ecol:512 * ecol + 512], op[:])
                        else:
                            nc.scalar.copy(ot[:, 512 * ecol:512 * ecol + 512], op[:])
                    nc.sync.dma_start(out_d[b * T + r0:b * T + r0 + 128, :], ot[:])

            pending = None
            for b in range(B):
                for j in range(NTQ):
                    qTc = emit_proj(b, j)
                    rc = emit_attn_core(b, j, qTc)
                    if pending is not None:
                        emit_norm_oproj(*pending)
                    pending = (b, j, rc)
            emit_norm_oproj(*pending)

    nc.compile()
    return nc


def _host_consts():
    inv = ROPE_BASE ** (-np.arange(32, dtype=np.float64) / 32.0)
    ang = np.outer(inv, np.arange(T, dtype=np.float64))          # [32, T]
    cos64 = np.tile(np.cos(ang), (2, 1)).astype(np.float32)      # [64, T]
    sin32 = np.sin(ang)
    sinS = np.concatenate([-sin32, sin32], axis=0).astype(np.float32)  # [64, T]
    cmask = np.triu(np.ones((128, 128))).astype(BF16NP)          # valid iff p <= j
    return cos64, sinS, cmask


def kernel(x, Wq, Wk, Wv, Wo):
    global _compiled, LAST_RESULT
    if _compiled is None:
        _compiled = _build()
    nc = _compiled

    xtb = np.ascontiguousarray(
        np.asarray(x, dtype=np.float32).reshape(B * T, E).T).astype(BF16NP)
    cos64, sinS, cmask = _host_consts()
    in_maps = []
    for c in range(NCORES):
        wkv = np.concatenate([Wk[:, D * c:D * (c + 1)], Wv[:, D * c:D * (c + 1)]],
                             axis=1)
        in_maps.append({
            "xT": xtb,
            "wq": np.ascontiguousarray(Wq[:, QH * c:QH * (c + 1)]).astype(BF16NP),
            "wkv": np.ascontiguousarray(wkv).astype(BF16NP),
            "wo": np.ascontiguousarray(Wo[QH * c:QH * (c + 1), :]).astype(BF16NP),
            "cos": cos64,
            "sin": sinS,
            "cmask": cmask,
        })
    trace = os.environ.get("GQA_TRACE", "0") == "1"
    res = run_bass_kernel_spmd(nc, in_maps, core_ids=list(range(NCORES)), trace=trace)
    LAST_RESULT = res
    acc = np.zeros((B * T, E), np.float32)
    for r in res.results:
        acc += np.asarray(r["out"]).astype(np.float32)
    return acc.reshape(B, T, E)


# revision 18
# speedup vs baseline: 1.4707x; 1.0228x over previous
"""GQA forward (B=2,T=2048,E=2048,H=32,HKV=8,D=64, RoPE, causal) on 8 trn2 cores.

Sharding: tensor-parallel over kv-heads. Core c owns kv-head c and q-heads
4c..4c+3 (columns 256c:256c+256 of Wq, 64c:64c+64 of Wk/Wv, rows
256c:256c+256 of Wo). Each core computes its heads' attention for both
batches plus the partial o-projection y_c @ Wo_c; the host sums the 8
partials.

v2 layout/schedule (vs v1 baseline):
  - x^T is pre-transposed on the host ([E, B*T] bf16) -> plain wide DMAs,
    no on-chip DMA transposes for x.
  - K and V projections packed into one [E,128] stationary (K^T rows 0:64,
    V^T rows 64:128 of one psum).
  - exp() batched over pairs of key blocks ([128,1024] psum reads).
  - softmax denominators for the 4 groups of a tq-chunk collected into one
    [4,512] tile via tiny DMAs; ONE reciprocal per (b,chunk) instead of 32
    pathological [1,512] reciprocals.
  - y kept unnormalized in SBUF; bcast+normalize+o-projection emitted one
    chunk late so the PE never waits on the reciprocal chain.
  - causal corner masks on gpsimd; copies split DVE/Act to balance engines.
"""
import os

import numpy as np
import ml_dtypes

import concourse.mybir as mybir
import concourse.tile as tile
from concourse import bacc
from concourse.bass_utils import run_bass_kernel_spmd

F32 = mybir.dt.float32
BF16 = mybir.dt.bfloat16
AF = mybir.ActivationFunctionType
BF16NP = ml_dtypes.bfloat16

B, T, E = 2, 2048, 2048
H, HKV, D = 32, 8, 64
G = H // HKV          # q heads per kv head (= per core)
NCORES = 8
QH = G * D            # 256 q cols per core
ECH = E // 128        # 16 contraction chunks
TQC = 512             # tq chunk width
NTQ = T // TQC        # 4
NSB = T // 128        # 16 key blocks per batch
ROPE_BASE = 10000.0

_compiled = None
LAST_RESULT = None


def _build():
    nc = bacc.Bacc(None, target_bir_lowering=False, debug=False)

    xt_d = nc.declare_dram_parameter("xT", [E, B * T], BF16, isOutput=False)
    wq_d = nc.declare_dram_parameter("wq", [E, QH], BF16, isOutput=False)
    wkv_d = nc.declare_dram_parameter("wkv", [E, 2 * D], BF16, isOutput=False)
    wo_d = nc.declare_dram_parameter("wo", [QH, E], BF16, isOutput=False)
    cos_d = nc.declare_dram_parameter("cos", [64, T], F32, isOutput=False)
    sin_d = nc.declare_dram_parameter("sin", [64, T], F32, isOutput=False)
    cm_d = nc.declare_dram_parameter("cmask", [128, 128], BF16, isOutput=False)
    out_d = nc.declare_dram_parameter("out", [B * T, E], BF16, isOutput=True)

    with tile.TileContext(nc) as tc:
        with (
            tc.tile_pool(name="const", bufs=1) as cp,
            tc.tile_pool(name="acts", bufs=1) as ac,
            tc.tile_pool(name="xT", bufs=2) as xp,
            tc.tile_pool(name="work", bufs=2) as wp,
            tc.tile_pool(name="ps1", bufs=2, space="PSUM") as ps1,   # 2 banks
            tc.tile_pool(name="psS", bufs=2, space="PSUM") as psS,   # 4 banks
            tc.tile_pool(name="psY", bufs=2, space="PSUM") as psY,   # 2 banks
        ):
            # weights needed first on the SP queue; the rest on gpsimd's queue
            # so the first projection isn't serialized behind them.
            wq_sb = cp.tile([128, ECH, QH], BF16)
            nc.sync.dma_start(wq_sb[:], wq_d[:, :].rearrange("(c p) n -> p c n", p=128))
            wkv_sb = cp.tile([128, ECH, 2 * D], BF16)
            nc.sync.dma_start(wkv_sb[:], wkv_d[:, :].rearrange("(c p) n -> p c n", p=128))
            cos_sb = cp.tile([64, T], F32)
            nc.gpsimd.dma_start(cos_sb[:], cos_d[:, :])
            sin_sb = cp.tile([64, T], F32)
            nc.gpsimd.dma_start(sin_sb[:], sin_d[:, :])
            cm_sb = cp.tile([128, 128], BF16)
            nc.gpsimd.dma_start(cm_sb[:], cm_d[:, :])
            wo_sb = cp.tile([128, 2, E], BF16)
            nc.gpsimd.dma_start(wo_sb[:], wo_d[:, :].rearrange("(c p) n -> p c n", p=128))
            ones_sb = cp.tile([1, D], BF16)
            nc.vector.memset(ones_sb[:], 1.0)

            kT, vS = {}, {}
            for b in range(B):
                kT[b] = ac.tile([64, T], BF16, name=f"kT{b}", tag=f"kT{b}")
                for i in range(NSB):
                    vS[b, i] = ac.tile([128, D + 1], BF16, name=f"vS{b}_{i}",
                                       tag=f"vS{b}_{i}")
                    nc.vector.memset(vS[b, i][:, D:D + 1], 1.0)
            # unnormalized y (head-pair-major) and normalized y; column ranges
            # are disjoint across (b, j) so two shared tiles each suffice.
            yU, yN = {}, {}
            for hp in range(2):
                yU[hp] = ac.tile([128, B * T], BF16, name=f"yU{hp}", tag=f"yU{hp}")
                yN[hp] = ac.tile([128, B * T], BF16, name=f"yN{hp}", tag=f"yN{hp}")

            def rope_store(dst, src, p0, tc0):
                # dst[bf16][0:64, TQC] <- rope(psum_f32[p0:p0+64, TQC]); DVE
                # (rotate-half needs partition-shifted APs: DVE-only)
                t1 = wp.tile([64, TQC], F32, tag="t1")
                t2 = wp.tile([64, TQC], F32, tag="t2")
                nc.vector.tensor_mul(t1[:, :], src[p0:p0 + 64, :],
                                     cos_sb[0:64, tc0:tc0 + TQC])
                nc.vector.tensor_mul(t2[0:32, :], src[p0 + 32:p0 + 64, :],
                                     sin_sb[0:32, tc0:tc0 + TQC])
                nc.vector.tensor_mul(t2[32:64, :], src[p0:p0 + 32, :],
                                     sin_sb[32:64, tc0:tc0 + TQC])
                nc.vector.tensor_add(dst, t1[:, :], t2[:, :])

            def emit_proj(b, j):
                rows0 = b * T + j * TQC
                tc0 = j * TQC
                xT_t = xp.tile([128, ECH, TQC], BF16, tag="xT")
                nc.sync.dma_start(
                    xT_t[:],
                    xt_d[:, rows0:rows0 + TQC].rearrange("(c p) n -> p c n", p=128))
                qTc = [wp.tile([64, TQC], BF16, tag=f"qT{g}", name=f"qT{g}")
                       for g in range(G)]
                for hp in range(2):
                    qp = ps1.tile([128, TQC], F32, tag="p1")
                    for ec in range(ECH):
                        nc.tensor.matmul(qp[:],
                                         wq_sb[:, ec, 128 * hp:128 * hp + 128],
                                         xT_t[:, ec, :],
                                         start=(ec == 0), stop=(ec == ECH - 1))
                    rope_store(qTc[2 * hp][:, :], qp, 0, tc0)
                    rope_store(qTc[2 * hp + 1][:, :], qp, 64, tc0)
                kvp = psY.tile([128, TQC], F32, tag="y")
                for ec in range(ECH):
                    nc.tensor.matmul(kvp[:], wkv_sb[:, ec, :], xT_t[:, ec, :],
                                     start=(ec == 0), stop=(ec == ECH - 1))
                rope_store(kT[b][:, tc0:tc0 + TQC], kvp, 0, tc0)
                vT = wp.tile([64, TQC], BF16, tag="vT")
                for tb in range(4):
                    sl = slice(tb * 128, (tb + 1) * 128)
                    nc.scalar.copy(vT[:, sl], kvp[64:128, sl])
                    nc.sync.dma_start_transpose(
                        vS[b, j * 4 + tb][:, 0:D], vT[:, sl])
                return qTc

            def emit_attn_core(b, j, qTc):
                """scores+exp+PV for all 4 groups; collect denoms; recip."""
                tc0 = j * TQC
                # denom rows parked at quadrant-aligned partitions 0/32/64/96
                # (engine APs cannot start at arbitrary partitions)
                den = wp.tile([128, TQC], F32, tag="den")
                rc = wp.tile([128, TQC], BF16, tag="rc")
                for g in range(G):
                    hp, h1 = g // 2, (g % 2) * 64
                    yp = psY.tile([65, TQC], F32, tag="y")
                    first = True
                    # off-diagonal full key blocks, in pairs
                    for p in range(2 * j):
                        sp2 = psS.tile([128, 2, TQC], F32, tag="s2")
                        pt2 = wp.tile([128, 2, TQC], BF16, tag="p2", bufs=3)
                        for u in range(2):
                            sb = 2 * p + u
                            nc.tensor.matmul(sp2[:, u, :],
                                             kT[b][:, 128 * sb:128 * sb + 128],
                                             qTc[g][:, :],
                                             start=True, stop=True)
                        nc.scalar.activation(pt2[:, :, :], sp2[:, :, :],
                                             AF.Exp, scale=0.125)
                        for u in range(2):
                            nc.tensor.matmul(yp[:, :], vS[b, 2 * p + u][:, :],
                                             pt2[:, u, :],
                                             start=first, stop=False,
                                             skip_group_check=True)
                            first = False
                    # diagonal 4 blocks (partial widths), 2 per psum pair-tile
                    for kk in range(0, 4, 2):
                        sp2 = psS.tile([128, 2, TQC], F32, tag="s2")
                        pt2 = wp.tile([128, 2, TQC], BF16, tag="p2", bufs=3)
                        for u in range(2):
                            k = kk + u
                            j0 = 128 * k
                            nc.tensor.matmul(sp2[:, u, j0:TQC],
                                             kT[b][:, 128 * (4 * j + k):128 * (4 * j + k) + 128],
                                             qTc[g][:, j0:TQC],
                                             start=True, stop=True)
                        for u in range(2):
                            k = kk + u
                            j0 = 128 * k
                            nc.scalar.activation(pt2[:, u, j0:TQC], sp2[:, u, j0:TQC],
                                                 AF.Exp, scale=0.125)
                            nc.gpsimd.tensor_mul(pt2[:, u, j0:j0 + 128],
                                                 pt2[:, u, j0:j0 + 128], cm_sb[:])
                            nc.tensor.matmul(yp[:, j0:TQC], vS[b, 4 * j + k][:, :],
                                             pt2[:, u, j0:TQC],
                                             start=(first and k == 0), stop=(k == 3),
                                             skip_group_check=True)
                    # free psum fast: unnormalized y -> SBUF, denom row -> collector
                    nc.vector.tensor_copy(yU[hp][h1:h1 + 64, b * T + tc0:b * T + tc0 + TQC],
                                          yp[0:64, :])
                    nc.vector.tensor_copy(den[32 * g:32 * g + 1, :], yp[64:65, :])
                with nc.allow_low_precision(reason="softmax denom bcast via bf16 matmul"):
                    nc.vector.reciprocal(rc[:, :], den[:, :])
                return rc

            def emit_norm_oproj(b, j, rc):
                tc0 = j * TQC
                c0 = b * T + tc0
                rcr = wp.tile([1, 4, TQC], BF16, tag="rcr")
                for g in range(G):
                    nc.sync.dma_start(rcr[0:1, g, :], rc[32 * g:32 * g + 1, :])
                for g in range(G):
                    hp, h1 = g // 2, (g % 2) * 64
                    bc = ps1.tile([64, TQC], F32, tag="p1")
                    nc.tensor.matmul(bc[:], ones_sb[:], rcr[0:1, g, :],
                                     start=True, stop=True)
                    nc.vector.tensor_mul(yN[hp][h1:h1 + 64, c0:c0 + TQC],
                                         yU[hp][h1:h1 + 64, c0:c0 + TQC], bc[:])
                for tb in range(4):
                    r0 = tc0 + tb * 128
                    ot = wp.tile([128, E], BF16, tag="ot")
                    for ecol in range(4):
                        op = ps1.tile([128, TQC], F32, tag="p1")
                        for hc in range(2):
                            nc.tensor.matmul(op[:], yN[hc][:, b * T + r0:b * T + r0 + 128],
                                             wo_sb[:, hc, 512 * ecol:512 * ecol + 512],
                                             start=(hc == 0), stop=(hc == 1))
                        if ecol % 2 == 0:
                            nc.vector.tensor_copy(ot[:, 512 * ecol:512 * ecol + 512], op[:])
                        else:
                            nc.scalar.copy(ot[:, 512 * ecol:512 * ecol + 512], op[:])
                    nc.gpsimd.dma_start(out_d[b * T + r0:b * T + r0 + 128, :], ot[:])

            # Pipeline: norm+o-proj of chunk j-1 is emitted after proj of
            # chunk j, so the PE never waits on the reciprocal/bcast chain.
            pending = None
            for b in range(B):
                for j in range(NTQ):
                    qTc = emit_proj(b, j)
                    if pending is not None:
                        emit_norm_oproj(*pending)
                    rc = emit_attn_core(b, j, qTc)
                    pending = (b, j, rc)
            emit_norm_oproj(*pending)

    nc.compile()
    return nc


def _host_consts():
    inv = ROPE_BASE ** (-np.arange(32, dtype=np.float64) / 32.0)
    ang = np.outer(inv, np.arange(T, dtype=np.float64))          # [32, T]
    cos64 = np.tile(np.cos(ang), (2, 1)).astype(np.float32)      # [64, T]
    sin32 = np.sin(ang)
    sinS = np.concatenate([-sin32, sin32], axis=0).astype(np.float32)  # [64, T]
    cmask = np.triu(np.ones((128, 128))).astype(BF16NP)          # valid iff p <= j
    return cos64, sinS, cmask


def kernel(x, Wq, Wk, Wv, Wo):
    global _compiled, LAST_RESULT
    if _compiled is None:
        _compiled = _build()
    nc = _compiled

    xtb = np.ascontiguousarray(
        np.asarray(x, dtype=np.float32).reshape(B * T, E).T).astype(BF16NP)
    cos64, sinS, cmask = _host_consts()
    in_maps = []
    for c in range(NCORES):
        wkv = np.concatenate([Wk[:, D * c:D * (c + 1)], Wv[:, D * c:D * (c + 1)]],
                             axis=1)
        in_maps.append({
            "xT": xtb,
            "wq": np.ascontiguousarray(Wq[:, QH * c:QH * (c + 1)]).astype(BF16NP),
            "wkv": np.ascontiguousarray(wkv).astype(BF16NP),
            "wo": np.ascontiguousarray(Wo[QH * c:QH * (c + 1), :]).astype(BF16NP),
            "cos": cos64,
            "sin": sinS,
            "cmask": cmask,
        })
    trace = os.environ.get("GQA_TRACE", "0") == "1"
    res = run_bass_kernel_spmd(nc, in_maps, core_ids=list(range(NCORES)), trace=trace)
    LAST_RESULT = res
    acc = np.zeros((B * T, E), np.float32)
    for r in res.results:
        acc += np.asarray(r["out"]).astype(np.float32)
    return acc.reshape(B, T, E)
